# revision 13
# baseline (speedup 1.0000x reference)
"""DressedQuantumNet on 8 TRN2 NeuronCores (pure data parallel).

Math: pre-net angles th = X @ pre_w.T + pre_b.  The quantum circuit after
the batch-dependent RY(th) layer is a FIXED unitary V (it only depends on
q_weights), and the initial state is the product state
psi = kron_w [a_w, b_w] with a=(cos(th/2)-sin(th/2))/sqrt(2),
b=(cos(th/2)+sin(th/2))/sqrt(2) (real).  Hence

  <Z_w> = psi^T Re(V^H Z_w V) psi
  out_k = psi^T C_k psi + post_b_k,   C_k = sum_w post_w[k,w] Re(V^H Z_w V)

and since (u u^T) is affine in (sin th, cos th), the whole quadratic form
collapses to an 81-coefficient multilinear polynomial in
v_w = [1, sin th_w, cos th_w]:

  out_k = sum_{m in 3^4} T_k[m] * prod_w v_w[m_w]

T_k is precomputed on host (tiny), so the device only computes the big
[B,512]@[512,4] matmul, sin/cos, and a few batched elementwise products.

Device layout (per core, batch on SBUF partitions):
  - DMA 2 MiB chunks of X; PE-transposes 128x128 blocks (fp32 has no DMA
    transpose) -> matmul with pre-transposed weights -> angles in PSUM.
  - angles + bias staged per group of G row-tiles; sin/cos via ScalarE
    Sin LUT after range-wrap into [-pi, pi] on DVE.
  - kron products + T contraction on DVE; results DMA'd out.
"""

from contextlib import ExitStack

import numpy as np

import concourse.bass as bass
import concourse.bacc as bacc_mod
import concourse.mybir as mybir
from concourse.bass_utils import run_bass_kernel_spmd
from concourse.tile import TileContext

N_CORES = 8
B_TOTAL = 65536
F_IN = 512
ROWS = B_TOTAL // N_CORES   # 8192 rows per core
P = 128
N_TILES = ROWS // P         # 64 row-tiles
G = 8                       # row-tiles per quantum-stage group
BIG = 8                     # row-tiles per input DMA (2 MiB)
N_GROUPS = N_TILES // G     # 4
DMAS_PER_GROUP = G // BIG   # 2

F32 = mybir.dt.float32
PI = float(np.pi)

N_QUBITS, VAR_DEPTH = 4, 3


# ----------------------------------------------------------------- host math
def _gate_1q(g, w):
    ops = [np.eye(2, dtype=complex)] * N_QUBITS
    ops[w] = g
    U = ops[0]
    for i in range(1, N_QUBITS):
        U = np.kron(U, ops[i])
    return U


def _bit(i, w):  # wire 0 = most significant
    return (i >> (N_QUBITS - 1 - w)) & 1


def _cnot(c, t):
    M = np.zeros((16, 16), dtype=complex)
    for i in range(16):
        j = i ^ (1 << (N_QUBITS - 1 - t)) if _bit(i, c) else i
        M[j, i] = 1.0
    return M


def _ry(theta):
    c, s = np.cos(theta / 2), np.sin(theta / 2)
    return np.array([[c, -s], [s, c]], dtype=complex)


def _rz(theta):
    ph = np.exp(1j * theta / 2)
    return np.array([[np.conj(ph), 0], [0, ph]], dtype=complex)


def _fixed_unitary(qw):
    V = np.eye(16, dtype=complex)

    def app(Gm):
        nonlocal V
        V = Gm @ V

    def entangle():
        app(_cnot(0, 1)); app(_cnot(2, 3)); app(_cnot(1, 2))

    for k in range(VAR_DEPTH):
        entangle()
        for w in range(N_QUBITS):
            app(_gate_1q(_ry(qw[k, w]), w))
        for w in range(N_QUBITS):
            app(_gate_1q(_rz(qw[k, w]), w))
    for k in range(VAR_DEPTH):
        entangle()
        for w in range(N_QUBITS):
            app(_gate_1q(_ry(qw[k, w]), w))
        for w in range(N_QUBITS):
            app(_gate_1q(_rz(qw[3 + k, w]), w))
    entangle()
    return V


def _build_T(q_weights, post_w, post_b):
    """[2, 81] coefficients; post_b folded into the constant term."""
    V = _fixed_unitary(np.asarray(q_weights, dtype=np.float64))
    E = np.zeros((3, 2, 2))
    E[0] = [[0.5, 0.0], [0.0, 0.5]]
    E[1] = [[-0.5, 0.0], [0.0, 0.5]]
    E[2] = [[0.0, 0.5], [0.5, 0.0]]
    Ts = []
    for k in range(2):
        C = np.zeros((16, 16), dtype=complex)
        for w in range(N_QUBITS):
            z = np.array([1.0 - 2.0 * _bit(i, w) for i in range(16)])
            C += post_w[k, w] * (V.conj().T @ np.diag(z) @ V)
        A = C.real.reshape([2] * 8)
        T = np.einsum("abcdefgh,iae,jbf,kcg,ldh->ijkl", A, E, E, E, E)
        T = T.reshape(81).copy()
        T[0] += post_b[k]
        Ts.append(T)
    return np.stack(Ts).astype(np.float32)  # [2, 81]


# ------------------------------------------------------------- device kernel
def build_bass(rows=ROWS, g_tiles=G):
    n_tiles = rows // P
    n_groups = n_tiles // g_tiles
    gb = g_tiles * P          # batch rows per group
    assert n_groups * g_tiles == n_tiles

    nc = bacc_mod.Bacc(None, target_bir_lowering=False)
    FP16 = mybir.dt.float16
    FP8 = mybir.dt.float8e5
    # host-packed: htp[g, p, k, b] so each group slab is one contiguous block
    ht_d = nc.dram_tensor("htp", [n_groups, P, 4, gb], FP16, kind="ExternalInput")
    lt_d = nc.dram_tensor("ltp", [n_groups, P, 4, gb], FP8, kind="ExternalInput")
    whl_d = nc.dram_tensor("whl", [P, 32], FP16, kind="ExternalInput")
    w8_d = nc.dram_tensor("w8", [P, 16], FP8, kind="ExternalInput")
    tc_d = nc.dram_tensor("tcoef", [P, 162], F32, kind="ExternalInput")
    bi_d = nc.dram_tensor("bias2", [P, 2, 4], F32, kind="ExternalInput")
    # out_dev[p, t, k] = out[t*128 + p, k]; host unscrambles
    out_d = nc.dram_tensor("out", [P, n_tiles, 2], F32, kind="ExternalOutput")

    with TileContext(nc) as tc, ExitStack() as ctx:
        const = ctx.enter_context(tc.tile_pool(name="const", bufs=1))
        whl = const.tile([P, 32], FP16)
        nc.scalar.dma_start(whl, whl_d[:])
        w8c = const.tile([P, 16], FP8)
        nc.scalar.dma_start(w8c, w8_d[:])
        tco = const.tile([P, 162], F32)
        nc.scalar.dma_start(tco, tc_d[:])
        bia = const.tile([P, 2, 4], F32)
        nc.scalar.dma_start(bia, bi_d[:])
        npi = const.tile([P, 1], F32)
        nc.vector.memset(npi, -PI)

        xp = ctx.enter_context(tc.tile_pool(name="xin", bufs=3))
        angp = ctx.enter_context(tc.tile_pool(name="angp", bufs=2, space="PSUM"))
        stg = ctx.enter_context(tc.tile_pool(name="stg", bufs=2))
        scr = ctx.enter_context(tc.tile_pool(name="scr", bufs=2))
        vvp = ctx.enter_context(tc.tile_pool(name="vv", bufs=2))
        wp = ctx.enter_context(tc.tile_pool(name="wpair", bufs=2))
        tqp = ctx.enter_context(tc.tile_pool(name="tq", bufs=2))
        qkp = ctx.enter_context(tc.tile_pool(name="qk", bufs=2))
        rp = ctx.enter_context(tc.tile_pool(name="res", bufs=2))

        resall = rp.tile([P, n_tiles, 2], F32)
        for g in range(n_groups):
            ht_sb = xp.tile([P, 4, gb], FP16, tag="ht")
            lt_sb = xp.tile([P, 4, gb], FP8, tag="lt")
            nc.sync.dma_start(ht_sb, ht_d[g])
            nc.sync.dma_start(lt_sb, lt_d[g])
            # one PSUM bank holds the whole group's angles: [p, t, 8]
            ang = angp.tile([P, g_tiles, 8], F32)
            for t in range(g_tiles):
                bs = t * P
                for k in range(4):
                    nc.tensor.matmul(
                        ang[:, t, :],
                        ht_sb[:, k, bs:bs + P],
                        whl[:, 8 * k:8 * k + 8],
                        start=(k == 0), stop=False,
                    )
                for k in range(4):
                    nc.tensor.matmul(
                        ang[:, t, 0:4],
                        lt_sb[:, k, bs:bs + P],
                        w8c[:, 4 * k:4 * k + 4],
                        start=False, stop=(k == 3),
                    )
            # theta_raw[p,t,w] = H-part + L-part (one strided PSUM reduce)
            sc_in = stg.tile([P, g_tiles, 4], F32)
            nc.vector.tensor_reduce(
                sc_in,
                ang.rearrange("p t (two w) -> p t w two", two=2),
                axis=mybir.AxisListType.X, op=mybir.AluOpType.add,
            )

            # ---- quantum stage for this group (all free-dim elementwise) ---
            # th[:, :, 0, :] = raw + pre_b + pi    (sin plane)
            # th[:, :, 1, :] = raw + pre_b + 3pi/2 (cos plane)
            th = stg.tile([P, g_tiles, 2, 4], F32, tag="th")
            nc.vector.tensor_add(
                th,
                sc_in.unsqueeze(2).broadcast_to([P, g_tiles, 2, 4]),
                bia.unsqueeze(1).broadcast_to([P, g_tiles, 2, 4]),
            )
            # range-wrap into [-pi, pi] (|theta| << 3pi, one wrap is exact)
            m1 = scr.tile([P, g_tiles, 2, 4], F32, tag="m1")
            a1 = scr.tile([P, g_tiles, 2, 4], F32, tag="a1")
            m2 = scr.tile([P, g_tiles, 2, 4], F32, tag="m2")
            a2 = scr.tile([P, g_tiles, 2, 4], F32, tag="a2")
            nc.vector.tensor_scalar(
                m1, th, PI, -2.0 * PI,
                op0=mybir.AluOpType.is_gt, op1=mybir.AluOpType.mult,
            )
            nc.vector.tensor_add(a1, th, m1)
            nc.vector.tensor_scalar(
                m2, a1, -PI, 2.0 * PI,
                op0=mybir.AluOpType.is_lt, op1=mybir.AluOpType.mult,
            )
            nc.vector.tensor_add(a2, a1, m2)
            # v = [1, sin, cos] per wire: vv[p, g, m, w]
            vv = vvp.tile([P, g_tiles, 3, 4], F32)
            nc.vector.memset(vv[:, :, 0, :], 1.0)
            nc.scalar.activation(
                vv[:, :, 1:3, :], a2, mybir.ActivationFunctionType.Sin
            )

            # w01[m0,m1] = v0[m0]*v1[m1]; w23[m2,m3] = v2[m2]*v3[m3]
            wpair = wp.tile([P, g_tiles, 2, 3, 3], F32)
            nc.vector.tensor_mul(
                wpair[:, :, 0],
                vv[:, :, :, 0].unsqueeze(3).broadcast_to([P, g_tiles, 3, 3]),
                vv[:, :, :, 1].unsqueeze(2).broadcast_to([P, g_tiles, 3, 3]),
            )
            nc.vector.tensor_mul(
                wpair[:, :, 1],
                vv[:, :, :, 2].unsqueeze(3).broadcast_to([P, g_tiles, 3, 3]),
                vv[:, :, :, 3].unsqueeze(2).broadcast_to([P, g_tiles, 3, 3]),
            )
            w01 = wpair[:, :, 0].rearrange("p g a b -> p g (a b)")
            w23 = wpair[:, :, 1].rearrange("p g a b -> p g (a b)")

            res = resall[:, g * g_tiles:(g + 1) * g_tiles, :]
            # both quadratic forms at once, (k,m01) merged into one 18-dim so
            # every AP stays within the ISA's 3-free-dim limit:
            # tq[p,g,km,m23] = w23[m23] * T[km, m23]
            tq = tqp.tile([P, g_tiles, 18, 9], F32, tag="tq")
            nc.gpsimd.tensor_mul(
                tq,
                w23.unsqueeze(2).broadcast_to([P, g_tiles, 18, 9]),
                tco[:, 0:162].rearrange("p (km b) -> p km b", b=9)
                   .unsqueeze(1).broadcast_to([P, g_tiles, 18, 9]),
            )
            qk = qkp.tile([P, g_tiles, 18], F32, tag="qk")
            nc.vector.tensor_reduce(
                qk, tq, axis=mybir.AxisListType.X, op=mybir.AluOpType.add
            )
            sk = qkp.tile([P, g_tiles, 2, 9], F32, tag="sk")
            nc.vector.tensor_mul(
                sk,
                qk.rearrange("p g (k m) -> p g k m", m=9),
                w01.unsqueeze(2).broadcast_to([P, g_tiles, 2, 9]),
            )
            nc.vector.tensor_reduce(
                res, sk, axis=mybir.AxisListType.X, op=mybir.AluOpType.add
            )
        nc.scalar.dma_start(out_d[:], resall)

    nc.finalize()
    return nc


_NC_CACHE = {}


def _get_nc(rows=ROWS):
    if rows not in _NC_CACHE:
        _NC_CACHE[rows] = build_bass(rows=rows)
    return _NC_CACHE[rows]


def _host_consts(pre_w, pre_b, q_weights, post_w, post_b):
    import ml_dtypes
    pre_w = np.asarray(pre_w, dtype=np.float32)
    wh = pre_w.astype(np.float16)
    wl = (pre_w - wh.astype(np.float32)).astype(np.float16)
    # whl[f_loc, 8k + j]: j<4 -> Wh[j, 128k+f_loc]; j>=4 -> Wl[j-4, 128k+f_loc]
    whl = np.zeros((P, 32), dtype=np.float16)
    w8c = np.zeros((P, 16), dtype=ml_dtypes.float8_e5m2)
    for k in range(4):
        whl[:, 8 * k:8 * k + 4] = wh.T[P * k:P * (k + 1)]
        whl[:, 8 * k + 4:8 * k + 8] = wl.T[P * k:P * (k + 1)]
        w8c[:, 4 * k:4 * k + 4] = pre_w.T[P * k:P * (k + 1)].astype(
            ml_dtypes.float8_e5m2)
    T = _build_T(
        np.asarray(q_weights, np.float64),
        np.asarray(post_w, np.float64),
        np.asarray(post_b, np.float64),
    )  # [2, 81] f32
    tco = np.broadcast_to(T.reshape(162), (P, 162)).copy()
    pb = np.asarray(pre_b, np.float64)
    b2 = np.stack([pb, pb + 0.5 * np.pi]).astype(np.float32)  # [2, 4]
    bias2 = np.broadcast_to(b2, (P, 2, 4)).copy()
    return {
        "whl": np.ascontiguousarray(whl),
        "w8": np.ascontiguousarray(w8c),
        "tcoef": np.ascontiguousarray(tco.astype(np.float32)),
        "bias2": np.ascontiguousarray(bias2),
    }


def _split_transpose(x, g_tiles=G):
    """x [ROWS, F] f32 -> (htp, ltp) each [n_groups, 128, 4, gb] bf16,
    htp[g, p, k, b] = bf16-hi(x)[g*gb + b, 128*k + p]."""
    import ml_dtypes
    rows = x.shape[0]
    gb = g_tiles * P
    n_groups = rows // gb
    h = x.astype(np.float16)
    l = (x - h.astype(np.float32)).astype(ml_dtypes.float8_e5m2)

    def pack(a):  # [rows, 512] -> [n_groups, 128, 4, gb]
        return np.ascontiguousarray(
            a.reshape(n_groups, gb, 4, P).transpose(0, 3, 2, 1)
        )

    return pack(h), pack(l)


def run(input_features, pre_w, pre_b, q_weights, post_w, post_b, **spmd_kwargs):
    x = np.asarray(input_features, dtype=np.float32)
    assert x.shape == (B_TOTAL, F_IN), x.shape
    consts = _host_consts(pre_w, pre_b, q_weights, post_w, post_b)
    in_maps = []
    for c in range(N_CORES):
        ht, lt = _split_transpose(x[c * ROWS:(c + 1) * ROWS])
        in_maps.append(dict(consts, htp=ht, ltp=lt))
    nc = _get_nc()
    r = run_bass_kernel_spmd(nc, in_maps, core_ids=list(range(N_CORES)), **spmd_kwargs)
    # out_dev[p, t, k] -> out[t*128 + p, k]
    out = np.concatenate(
        [r.results[c]["out"].transpose(1, 0, 2).reshape(ROWS, 2) for c in range(N_CORES)],
        axis=0,
    )
    return out.astype(np.float32), r


def kernel(input_features, pre_w, pre_b, q_weights, post_w, post_b):
    out, _ = run(input_features, pre_w, pre_b, q_weights, post_w, post_b)
    return out


# revision 14
# speedup vs baseline: 1.1999x; 1.1999x over previous
"""DressedQuantumNet on 8 TRN2 NeuronCores (pure data parallel).

Math: pre-net angles th = X @ pre_w.T + pre_b.  The quantum circuit after
the batch-dependent RY(th) layer is a FIXED unitary V (it only depends on
q_weights), and the initial state is the product state
psi = kron_w [a_w, b_w] with a=(cos(th/2)-sin(th/2))/sqrt(2),
b=(cos(th/2)+sin(th/2))/sqrt(2) (real).  Hence

  <Z_w> = psi^T Re(V^H Z_w V) psi
  out_k = psi^T C_k psi + post_b_k,   C_k = sum_w post_w[k,w] Re(V^H Z_w V)

and since (u u^T) is affine in (sin th, cos th), the whole quadratic form
collapses to an 81-coefficient multilinear polynomial in
v_w = [1, sin th_w, cos th_w]:

  out_k = sum_{m in 3^4} T_k[m] * prod_w v_w[m_w]

T_k is precomputed on host (tiny), so the device only computes the big
[B,512]@[512,4] matmul, sin/cos, and a few batched elementwise products.

Device layout (per core, batch on SBUF partitions):
  - DMA 2 MiB chunks of X; PE-transposes 128x128 blocks (fp32 has no DMA
    transpose) -> matmul with pre-transposed weights -> angles in PSUM.
  - angles + bias staged per group of G row-tiles; sin/cos via ScalarE
    Sin LUT after range-wrap into [-pi, pi] on DVE.
  - kron products + T contraction on DVE; results DMA'd out.
"""

from contextlib import ExitStack

import numpy as np

import concourse.bass as bass
import concourse.bacc as bacc_mod
import concourse.mybir as mybir
from concourse.bass_utils import run_bass_kernel_spmd
from concourse.tile import TileContext

N_CORES = 8
B_TOTAL = 65536
F_IN = 512
ROWS = B_TOTAL // N_CORES   # 8192 rows per core
P = 128
N_TILES = ROWS // P         # 64 row-tiles
G = 8                       # row-tiles per quantum-stage group
BIG = 8                     # row-tiles per input DMA (2 MiB)
N_GROUPS = N_TILES // G     # 4
DMAS_PER_GROUP = G // BIG   # 2

F32 = mybir.dt.float32
PI = float(np.pi)

N_QUBITS, VAR_DEPTH = 4, 3


# ----------------------------------------------------------------- host math
def _gate_1q(g, w):
    ops = [np.eye(2, dtype=complex)] * N_QUBITS
    ops[w] = g
    U = ops[0]
    for i in range(1, N_QUBITS):
        U = np.kron(U, ops[i])
    return U


def _bit(i, w):  # wire 0 = most significant
    return (i >> (N_QUBITS - 1 - w)) & 1


def _cnot(c, t):
    M = np.zeros((16, 16), dtype=complex)
    for i in range(16):
        j = i ^ (1 << (N_QUBITS - 1 - t)) if _bit(i, c) else i
        M[j, i] = 1.0
    return M


def _ry(theta):
    c, s = np.cos(theta / 2), np.sin(theta / 2)
    return np.array([[c, -s], [s, c]], dtype=complex)


def _rz(theta):
    ph = np.exp(1j * theta / 2)
    return np.array([[np.conj(ph), 0], [0, ph]], dtype=complex)


def _fixed_unitary(qw):
    V = np.eye(16, dtype=complex)

    def app(Gm):
        nonlocal V
        V = Gm @ V

    def entangle():
        app(_cnot(0, 1)); app(_cnot(2, 3)); app(_cnot(1, 2))

    for k in range(VAR_DEPTH):
        entangle()
        for w in range(N_QUBITS):
            app(_gate_1q(_ry(qw[k, w]), w))
        for w in range(N_QUBITS):
            app(_gate_1q(_rz(qw[k, w]), w))
    for k in range(VAR_DEPTH):
        entangle()
        for w in range(N_QUBITS):
            app(_gate_1q(_ry(qw[k, w]), w))
        for w in range(N_QUBITS):
            app(_gate_1q(_rz(qw[3 + k, w]), w))
    entangle()
    return V


def _build_T(q_weights, post_w, post_b):
    """[2, 81] coefficients; post_b folded into the constant term."""
    V = _fixed_unitary(np.asarray(q_weights, dtype=np.float64))
    E = np.zeros((3, 2, 2))
    E[0] = [[0.5, 0.0], [0.0, 0.5]]
    E[1] = [[-0.5, 0.0], [0.0, 0.5]]
    E[2] = [[0.0, 0.5], [0.5, 0.0]]
    Ts = []
    for k in range(2):
        C = np.zeros((16, 16), dtype=complex)
        for w in range(N_QUBITS):
            z = np.array([1.0 - 2.0 * _bit(i, w) for i in range(16)])
            C += post_w[k, w] * (V.conj().T @ np.diag(z) @ V)
        A = C.real.reshape([2] * 8)
        T = np.einsum("abcdefgh,iae,jbf,kcg,ldh->ijkl", A, E, E, E, E)
        T = T.reshape(81).copy()
        T[0] += post_b[k]
        Ts.append(T)
    return np.stack(Ts).astype(np.float32)  # [2, 81]


# ------------------------------------------------------------- device kernel
def build_bass(rows=ROWS, g_tiles=G):
    n_tiles = rows // P
    n_groups = n_tiles // g_tiles
    gb = g_tiles * P          # batch rows per group
    assert n_groups * g_tiles == n_tiles

    nc = bacc_mod.Bacc(None, target_bir_lowering=False)
    FP16 = mybir.dt.float16
    FP8 = mybir.dt.float8e5
    # host-packed: htp[g, p, k, b] so each group slab is one contiguous block
    ht_d = nc.dram_tensor("htp", [n_groups, P, 4, gb], FP16, kind="ExternalInput")
    lt_d = nc.dram_tensor("ltp", [n_groups, P, 4, gb], FP8, kind="ExternalInput")
    whl_d = nc.dram_tensor("whl", [P, 32], FP16, kind="ExternalInput")
    w8_d = nc.dram_tensor("w8", [P, 16], FP8, kind="ExternalInput")
    tc_d = nc.dram_tensor("tcoef", [P, 162], F32, kind="ExternalInput")
    bi_d = nc.dram_tensor("bias2", [P, 2, 4], F32, kind="ExternalInput")
    # out_dev[p, t, k] = out[t*128 + p, k]; host unscrambles
    out_d = nc.dram_tensor("out", [P, n_tiles, 2], F32, kind="ExternalOutput")

    with TileContext(nc) as tc, ExitStack() as ctx:
        const = ctx.enter_context(tc.tile_pool(name="const", bufs=1))
        whl = const.tile([P, 32], FP16)
        nc.scalar.dma_start(whl, whl_d[:])
        w8c = const.tile([P, 16], FP8)
        nc.scalar.dma_start(w8c, w8_d[:])
        tco = const.tile([P, 162], F32)
        nc.scalar.dma_start(tco, tc_d[:])
        bia = const.tile([P, 2, 4], F32)
        nc.scalar.dma_start(bia, bi_d[:])
        npi = const.tile([P, 1], F32)
        nc.vector.memset(npi, -PI)

        xp = ctx.enter_context(tc.tile_pool(name="xin", bufs=3))
        angp = ctx.enter_context(tc.tile_pool(name="angp", bufs=2, space="PSUM"))
        stg = ctx.enter_context(tc.tile_pool(name="stg", bufs=2))
        scr = ctx.enter_context(tc.tile_pool(name="scr", bufs=2))
        vvp = ctx.enter_context(tc.tile_pool(name="vv", bufs=2))
        wp = ctx.enter_context(tc.tile_pool(name="wpair", bufs=2))
        tqp = ctx.enter_context(tc.tile_pool(name="tq", bufs=2))
        qkp = ctx.enter_context(tc.tile_pool(name="qk", bufs=2))
        rp = ctx.enter_context(tc.tile_pool(name="res", bufs=2))

        resall = rp.tile([P, n_tiles, 2], F32)
        for g in range(n_groups):
            ht_sb = xp.tile([P, 4, gb], FP16, tag="ht")
            lt_sb = xp.tile([P, 4, gb], FP8, tag="lt")
            nc.sync.dma_start(ht_sb, ht_d[g])
            nc.sync.dma_start(lt_sb, lt_d[g])
            # one PSUM bank holds the whole group's angles: [p, t, 8]
            ang = angp.tile([P, g_tiles, 8], F32)
            for t in range(g_tiles):
                bs = t * P
                for k in range(4):
                    nc.tensor.matmul(
                        ang[:, t, :],
                        ht_sb[:, k, bs:bs + P],
                        whl[:, 8 * k:8 * k + 8],
                        start=(k == 0), stop=False,
                    )
                for k in range(4):
                    nc.tensor.matmul(
                        ang[:, t, 0:4],
                        lt_sb[:, k, bs:bs + P],
                        w8c[:, 4 * k:4 * k + 4],
                        start=False, stop=(k == 3),
                    )
            # theta_raw[p,t,w] = H-part + L-part (one strided PSUM reduce)
            sc_in = stg.tile([P, g_tiles, 4], F32)
            nc.vector.tensor_reduce(
                sc_in,
                ang.rearrange("p t (two w) -> p t w two", two=2),
                axis=mybir.AxisListType.X, op=mybir.AluOpType.add,
            )

            # ---- quantum stage for this group (all free-dim elementwise) ---
            # th[:, :, 0, :] = raw + pre_b + pi    (sin plane)
            # th[:, :, 1, :] = raw + pre_b + 3pi/2 (cos plane)
            th = stg.tile([P, g_tiles, 2, 4], F32, tag="th")
            nc.vector.tensor_add(
                th,
                sc_in.unsqueeze(2).broadcast_to([P, g_tiles, 2, 4]),
                bia.unsqueeze(1).broadcast_to([P, g_tiles, 2, 4]),
            )
            # range-wrap into [-pi, pi] (|theta| << 3pi, one wrap is exact)
            m1 = scr.tile([P, g_tiles, 2, 4], F32, tag="m1")
            a1 = scr.tile([P, g_tiles, 2, 4], F32, tag="a1")
            m2 = scr.tile([P, g_tiles, 2, 4], F32, tag="m2")
            a2 = scr.tile([P, g_tiles, 2, 4], F32, tag="a2")
            nc.vector.tensor_scalar(
                m1, th, PI, -2.0 * PI,
                op0=mybir.AluOpType.is_gt, op1=mybir.AluOpType.mult,
            )
            nc.vector.tensor_add(a1, th, m1)
            nc.vector.tensor_scalar(
                m2, a1, -PI, 2.0 * PI,
                op0=mybir.AluOpType.is_lt, op1=mybir.AluOpType.mult,
            )
            nc.vector.tensor_add(a2, a1, m2)
            # v = [1, sin, cos] per wire: vv[p, g, m, w]
            vv = vvp.tile([P, g_tiles, 3, 4], F32)
            nc.vector.memset(vv[:, :, 0, :], 1.0)
            nc.scalar.activation(
                vv[:, :, 1:3, :], a2, mybir.ActivationFunctionType.Sin
            )

            # w01[m0,m1] = v0[m0]*v1[m1]; w23[m2,m3] = v2[m2]*v3[m3]
            wpair = wp.tile([P, g_tiles, 2, 3, 3], F32)
            nc.vector.tensor_mul(
                wpair[:, :, 0],
                vv[:, :, :, 0].unsqueeze(3).broadcast_to([P, g_tiles, 3, 3]),
                vv[:, :, :, 1].unsqueeze(2).broadcast_to([P, g_tiles, 3, 3]),
            )
            nc.vector.tensor_mul(
                wpair[:, :, 1],
                vv[:, :, :, 2].unsqueeze(3).broadcast_to([P, g_tiles, 3, 3]),
                vv[:, :, :, 3].unsqueeze(2).broadcast_to([P, g_tiles, 3, 3]),
            )
            w01 = wpair[:, :, 0].rearrange("p g a b -> p g (a b)")
            w23 = wpair[:, :, 1].rearrange("p g a b -> p g (a b)")

            res = resall[:, g * g_tiles:(g + 1) * g_tiles, :]
            # both quadratic forms at once, (k,m01) merged into one 18-dim so
            # every AP stays within the ISA's 3-free-dim limit:
            # tq[p,g,km,m23] = w23[m23] * T[km, m23]
            tq = tqp.tile([P, g_tiles, 18, 9], F32, tag="tq")
            nc.vector.tensor_mul(
                tq,
                w23.unsqueeze(2).broadcast_to([P, g_tiles, 18, 9]),
                tco[:, 0:162].rearrange("p (km b) -> p km b", b=9)
                   .unsqueeze(1).broadcast_to([P, g_tiles, 18, 9]),
            )
            qk = qkp.tile([P, g_tiles, 18], F32, tag="qk")
            nc.vector.tensor_reduce(
                qk, tq, axis=mybir.AxisListType.X, op=mybir.AluOpType.add
            )
            sk = qkp.tile([P, g_tiles, 2, 9], F32, tag="sk")
            nc.vector.tensor_mul(
                sk,
                qk.rearrange("p g (k m) -> p g k m", m=9),
                w01.unsqueeze(2).broadcast_to([P, g_tiles, 2, 9]),
            )
            nc.vector.tensor_reduce(
                res, sk, axis=mybir.AxisListType.X, op=mybir.AluOpType.add
            )
        nc.scalar.dma_start(out_d[:], resall)

    nc.finalize()
    return nc


_NC_CACHE = {}


def _get_nc(rows=ROWS):
    if rows not in _NC_CACHE:
        _NC_CACHE[rows] = build_bass(rows=rows)
    return _NC_CACHE[rows]


def _host_consts(pre_w, pre_b, q_weights, post_w, post_b):
    import ml_dtypes
    pre_w = np.asarray(pre_w, dtype=np.float32)
    wh = pre_w.astype(np.float16)
    wl = (pre_w - wh.astype(np.float32)).astype(np.float16)
    # whl[f_loc, 8k + j]: j<4 -> Wh[j, 128k+f_loc]; j>=4 -> Wl[j-4, 128k+f_loc]
    whl = np.zeros((P, 32), dtype=np.float16)
    w8c = np.zeros((P, 16), dtype=ml_dtypes.float8_e5m2)
    for k in range(4):
        whl[:, 8 * k:8 * k + 4] = wh.T[P * k:P * (k + 1)]
        whl[:, 8 * k + 4:8 * k + 8] = wl.T[P * k:P * (k + 1)]
        w8c[:, 4 * k:4 * k + 4] = pre_w.T[P * k:P * (k + 1)].astype(
            ml_dtypes.float8_e5m2)
    T = _build_T(
        np.asarray(q_weights, np.float64),
        np.asarray(post_w, np.float64),
        np.asarray(post_b, np.float64),
    )  # [2, 81] f32
    tco = np.broadcast_to(T.reshape(162), (P, 162)).copy()
    pb = np.asarray(pre_b, np.float64)
    b2 = np.stack([pb, pb + 0.5 * np.pi]).astype(np.float32)  # [2, 4]
    bias2 = np.broadcast_to(b2, (P, 2, 4)).copy()
    return {
        "whl": np.ascontiguousarray(whl),
        "w8": np.ascontiguousarray(w8c),
        "tcoef": np.ascontiguousarray(tco.astype(np.float32)),
        "bias2": np.ascontiguousarray(bias2),
    }


def _split_transpose(x, g_tiles=G):
    """x [ROWS, F] f32 -> (htp, ltp) each [n_groups, 128, 4, gb] bf16,
    htp[g, p, k, b] = bf16-hi(x)[g*gb + b, 128*k + p]."""
    import ml_dtypes
    rows = x.shape[0]
    gb = g_tiles * P
    n_groups = rows // gb
    h = x.astype(np.float16)
    l = (x - h.astype(np.float32)).astype(ml_dtypes.float8_e5m2)

    def pack(a):  # [rows, 512] -> [n_groups, 128, 4, gb]
        return np.ascontiguousarray(
            a.reshape(n_groups, gb, 4, P).transpose(0, 3, 2, 1)
        )

    return pack(h), pack(l)


def run(input_features, pre_w, pre_b, q_weights, post_w, post_b, **spmd_kwargs):
    x = np.asarray(input_features, dtype=np.float32)
    assert x.shape == (B_TOTAL, F_IN), x.shape
    consts = _host_consts(pre_w, pre_b, q_weights, post_w, post_b)
    in_maps = []
    for c in range(N_CORES):
        ht, lt = _split_transpose(x[c * ROWS:(c + 1) * ROWS])
        in_maps.append(dict(consts, htp=ht, ltp=lt))
    nc = _get_nc()
    r = run_bass_kernel_spmd(nc, in_maps, core_ids=list(range(N_CORES)), **spmd_kwargs)
    # out_dev[p, t, k] -> out[t*128 + p, k]
    out = np.concatenate(
        [r.results[c]["out"].transpose(1, 0, 2).reshape(ROWS, 2) for c in range(N_CORES)],
        axis=0,
    )
    return out.astype(np.float32), r


def kernel(input_features, pre_w, pre_b, q_weights, post_w, post_b):
    out, _ = run(input_features, pre_w, pre_b, q_weights, post_w, post_b)
    return out
